# revision 5
# baseline (speedup 1.0000x reference)
"""TRN2 Bass kernel for nn_CodeSimilarityDetectionModel (GNN message passing).

Model (per graph pair, B=64 pairs, N=1024 nodes/graph, H=IN_DIM=128, C=4):
  h = 3-layer GCN encoder (shared weights) on each side
  S0 = softmax_row(Hs @ Ht^T)              [B, N, N]
  logits = MLP(|mean(Hs) - mean(Ht)|)      [B, C]

Distribution: batch-parallel over 8 NeuronCores, 8 graph pairs per core,
weights replicated, no collectives.

Device algorithm per graph-side:
  - Dense normalized adjacency A^T (with self loops, deg^-1/2 norms and
    duplicate-edge multiplicity baked into per-edge values) is built in
    SBUF as 8 blocks of [128 src x 1024 dst] bf16 via GPSIMD local_scatter
    from host-prepared padded CSR rows (values are exact bf16 roundings).
  - Layers: Y = (h W) in fp32 on PE;  Z^T = sum_m Y[m-blk]^T-matmuls with
    A^T blocks as the bf16 moving operand (PSUM accumulates in fp32);
    h_next^T = relu(Z^T + b) on ACT straight out of PSUM.
  - S0 logits: f32r x f32r matmuls of h3^T blocks; row-softmax via
    (DVE max-reduce, ACT exp with accumulate, two-pass exp with
    bias = -max - ln(sum)).
"""

import sys

sys.path.insert(0, "/opt/trn_rl_repo")

import numpy as np
import ml_dtypes

import concourse.bass as bass
import concourse.mybir as mybir
from concourse import bacc, bass_utils
from concourse.tile import TileContext
from concourse.masks import make_identity

# ---------------------------------------------------------------- constants
B, N, H, IN_DIM, DEG, C = 64, 1024, 128, 128, 16, 4
NCORES = 8
GPC = B // NCORES          # graph pairs per core (8)
NBLK = N // 128            # 8 row blocks per graph
PADW = 64                  # padded CSR width (max row degree; asserted)
F32 = mybir.dt.float32
F32R = mybir.dt.float32r
F16 = mybir.dt.float16
I16 = mybir.dt.int16

_BUILT = {}


# ---------------------------------------------------------------- host prep
def _prep_side(edge_index, core, g):
    """Padded CSR (by src) for graph `core*GPC+g`, with self loops, dup
    multiplicity and symmetric deg^-1/2 normalization baked into values.

    Returns (idx [NBLK,128,PADW] int16 with -1 padding,
             val [NBLK,128,PADW] bfloat16)."""
    bg = core * GPC + g
    e0 = bg * N * DEG
    src = (edge_index[0, e0:e0 + N * DEG] - bg * N).astype(np.int64)
    dst = (edge_index[1, e0:e0 + N * DEG] - bg * N).astype(np.int64)

    loop = np.arange(N, dtype=np.int64)
    dst_all = np.concatenate([dst, loop])
    deg = np.bincount(dst_all, minlength=N).astype(np.float64)
    dinv = (1.0 / np.sqrt(deg)).astype(np.float32)

    key = np.concatenate([src * N + dst, loop * N + loop])
    uniq, counts = np.unique(key, return_counts=True)
    usrc = (uniq // N).astype(np.int64)
    udst = (uniq % N).astype(np.int64)
    vals = counts.astype(np.float32) * dinv[usrc] * dinv[udst]

    starts = np.searchsorted(usrc, np.arange(N))
    pos = np.arange(len(usrc)) - starts[usrc]
    assert pos.max() < PADW, f"row degree {pos.max() + 1} exceeds PADW={PADW}"

    idx = np.full((N, PADW), -1, dtype=np.int16)
    val = np.zeros((N, PADW), dtype=np.float32)
    idx[usrc, pos] = udst.astype(np.int16)
    val[usrc, pos] = vals
    return (idx.reshape(NBLK, 128, PADW),
            val.reshape(NBLK, 128, PADW).astype(np.float16))


def _host_preprocess(inputs):
    """Slice + localize per-core inputs; CSR-ify edges. Returns in_maps."""
    x_s = np.asarray(inputs["x_s"], dtype=np.float32)
    x_t = np.asarray(inputs["x_t"], dtype=np.float32)
    ei_s = np.asarray(inputs["edge_index_s"])
    ei_t = np.asarray(inputs["edge_index_t"])

    weights = {}
    for k in ("W1", "W2", "W3", "Wc1", "Wc2"):
        weights[k] = np.ascontiguousarray(np.asarray(inputs[k], np.float32))
    for k in ("b1", "b2", "b3", "bc1", "bc2"):
        weights[k] = np.ascontiguousarray(np.asarray(inputs[k], np.float32))

    in_maps = []
    for core in range(NCORES):
        r0 = core * GPC * N
        m = dict(weights)
        m["x_s"] = np.ascontiguousarray(x_s[r0:r0 + GPC * N])
        m["x_t"] = np.ascontiguousarray(x_t[r0:r0 + GPC * N])
        for name, ei in (("s", ei_s), ("t", ei_t)):
            idxs = np.empty((GPC, NBLK, 128, PADW), np.int16)
            vals = np.empty((GPC, NBLK, 128, PADW), np.float16)
            for g in range(GPC):
                idxs[g], vals[g] = _prep_side(ei, core, g)
            m[f"ci_{name}"] = idxs
            m[f"cv_{name}"] = vals
        in_maps.append(m)
    return in_maps


# ---------------------------------------------------------------- device
def _build(nreps=1):
    """Build + compile the per-core Bass program. nreps>1 wraps the body in
    a For_i loop for wall-clock timing (work repeats identically)."""
    nc = bacc.Bacc("TRN2", target_bir_lowering=False, debug=False)

    x_d = {s: nc.dram_tensor(f"x_{s}", [GPC * N, IN_DIM], F32,
                             kind="ExternalInput") for s in "st"}
    ci_d = {s: nc.dram_tensor(f"ci_{s}", [GPC, NBLK, 128, PADW], I16,
                              kind="ExternalInput") for s in "st"}
    cv_d = {s: nc.dram_tensor(f"cv_{s}", [GPC, NBLK, 128, PADW], F16,
                              kind="ExternalInput") for s in "st"}
    W_d = [nc.dram_tensor(n, [H, H], F32, kind="ExternalInput")
           for n in ("W1", "W2", "W3")]
    b_d = [nc.dram_tensor(n, [H], F32, kind="ExternalInput")
           for n in ("b1", "b2", "b3")]
    Wc1_d = nc.dram_tensor("Wc1", [H, H], F32, kind="ExternalInput")
    bc1_d = nc.dram_tensor("bc1", [H], F32, kind="ExternalInput")
    Wc2_d = nc.dram_tensor("Wc2", [H, C], F32, kind="ExternalInput")
    bc2_d = nc.dram_tensor("bc2", [C], F32, kind="ExternalInput")

    s0_d = nc.dram_tensor("s0", [GPC * N, N], F32, kind="ExternalOutput")
    lg_d = nc.dram_tensor("lg", [GPC, C], F32, kind="ExternalOutput")

    with TileContext(nc) as tc:
        import contextlib
        ctx = contextlib.ExitStack()
        with ctx:
            cpool = ctx.enter_context(tc.tile_pool(name="consts", bufs=1))
            apool = ctx.enter_context(tc.tile_pool(name="amat", bufs=1))
            hpool = ctx.enter_context(tc.tile_pool(name="acts", bufs=2))
            iopool = ctx.enter_context(tc.tile_pool(name="io", bufs=2))
            spool = ctx.enter_context(tc.tile_pool(name="smax", bufs=3))
            psxy = ctx.enter_context(
                tc.tile_pool(name="psxy", bufs=2, space="PSUM"))
            psz = ctx.enter_context(
                tc.tile_pool(name="psz", bufs=1, space="PSUM"))
            pss = ctx.enter_context(
                tc.tile_pool(name="pss", bufs=1, space="PSUM"))

            # ---- constants
            ident = cpool.tile([128, 128], F32, tag="ident")
            make_identity(nc, ident[:])
            W_t = []
            b_t = []
            for l in range(3):
                wt = cpool.tile([H, H], F32, tag=f"W{l}")
                nc.sync.dma_start(out=wt[:], in_=W_d[l][:, :])
                W_t.append(wt)
                bt = cpool.tile([H, 1], F32, tag=f"b{l}")
                nc.sync.dma_start(out=bt[:], in_=b_d[l][:, None])
                b_t.append(bt)
            wc1 = cpool.tile([H, H], F32, tag="wc1")
            nc.sync.dma_start(out=wc1[:], in_=Wc1_d[:, :])
            wc2 = cpool.tile([H, C], F32, tag="wc2")
            nc.sync.dma_start(out=wc2[:], in_=Wc2_d[:, :])
            bc1 = cpool.tile([H, 1], F32, tag="bc1")
            nc.sync.dma_start(out=bc1[:], in_=bc1_d[:, None])
            bc2 = cpool.tile([C, 1], F32, tag="bc2")
            nc.sync.dma_start(out=bc2[:], in_=bc2_d[:, None])

            gs_t = {s: cpool.tile([H, GPC], F32, tag=f"gsum{s}",
                                  name=f"gsum_{s}") for s in "st"}

            def body():
                for g in range(GPC):
                    h3 = {}
                    for s in "st":
                        # -- load x rows for this graph, 8 col-blocks
                        xg = iopool.tile([128, NBLK * 128], F32, tag="xg")
                        nc.sync.dma_start(
                            out=xg[:].rearrange("p (nb h) -> p nb h", h=128),
                            in_=x_d[s][g * N:(g + 1) * N, :].rearrange(
                                "(nb p) h -> p nb h", p=128))
                        # -- transpose to x^T [h, n]
                        pxt = psxy.tile([128, NBLK * 128], F32, tag="xy")
                        for nb in range(NBLK):
                            nc.tensor.transpose(
                                out=pxt[:, nb * 128:(nb + 1) * 128],
                                in_=xg[:, nb * 128:(nb + 1) * 128],
                                identity=ident[:])
                        hT = hpool.tile([128, N], F32, tag=f"hT{s}")
                        nc.vector.tensor_copy(out=hT[:], in_=pxt[:])

                        # -- build A^T (normalized, with loops) via scatter
                        amat = apool.tile([128, NBLK * N], F16, tag=f"A{s}")
                        ci = iopool.tile([128, NBLK * PADW], I16, tag="ci")
                        cv = iopool.tile([128, NBLK * PADW], F16, tag="cv")
                        nc.sync.dma_start(
                            out=ci[:].rearrange("p (nb w) -> p nb w", w=PADW),
                            in_=ci_d[s][g].rearrange("nb p w -> p nb w"))
                        nc.sync.dma_start(
                            out=cv[:].rearrange("p (nb w) -> p nb w", w=PADW),
                            in_=cv_d[s][g].rearrange("nb p w -> p nb w"))
                        for mb in range(NBLK):
                            nc.gpsimd.local_scatter(
                                out_ap=amat[:, mb * N:(mb + 1) * N],
                                data_ap=cv[:, mb * PADW:(mb + 1) * PADW],
                                idxs_ap=ci[:, mb * PADW:(mb + 1) * PADW],
                                channels=128, num_elems=N, num_idxs=PADW)

                        # -- 3 GCN layers
                        for l in range(3):
                            py = psxy.tile([128, NBLK * 128], F32, tag="xy")
                            for nb in range(NBLK):
                                nc.tensor.matmul(
                                    out=py[:, nb * 128:(nb + 1) * 128],
                                    lhsT=hT[:, nb * 128:(nb + 1) * 128],
                                    rhs=W_t[l][:], start=True, stop=True)
                            ybf = hpool.tile([128, N], F16, tag="ybf")
                            nc.vector.tensor_copy(out=ybf[:], in_=py[:])
                            pz = psz.tile([128, N], F32, tag="z")
                            for mb in range(NBLK):
                                for cchunk in range(2):
                                    cs = slice(cchunk * 512, cchunk * 512 + 512)
                                    nc.tensor.matmul(
                                        out=pz[:, cs],
                                        lhsT=ybf[:, mb * 128:(mb + 1) * 128],
                                        rhs=amat[:, mb * N:(mb + 1) * N][:, cs],
                                        start=(mb == 0), stop=(mb == NBLK - 1))
                            if l < 2:
                                hT = hpool.tile([128, N], F32, tag=f"hT{s}")
                                nc.scalar.activation(
                                    out=hT[:], in_=pz[:],
                                    func=mybir.ActivationFunctionType.Relu,
                                    bias=b_t[l][:, 0:1])
                            else:
                                h3r = hpool.tile([128, N], F32R, tag=f"h3{s}")
                                nc.scalar.activation(
                                    out=h3r[:], in_=pz[:],
                                    func=mybir.ActivationFunctionType.Identity,
                                    bias=b_t[l][:, 0:1])
                                h3[s] = h3r
                        # -- graph embedding accumulation (sum over nodes)
                        nc.vector.tensor_reduce(
                            out=gs_t[s][:, g:g + 1],
                            in_=h3[s][:].bitcast(F32),
                            axis=mybir.AxisListType.X, op=mybir.AluOpType.add)

                    # -- S0 = row-softmax(Hs @ Ht^T) for this pair
                    for nb in range(NBLK):
                        ps = pss.tile([128, N], F32, tag="s")
                        for cchunk in range(2):
                            cs = slice(cchunk * 512, cchunk * 512 + 512)
                            nc.tensor.matmul(
                                out=ps[:, cs],
                                lhsT=h3["s"][:, nb * 128:(nb + 1) * 128],
                                rhs=h3["t"][:, cs],
                                start=True, stop=True)
                        negmax = spool.tile([128, 1], F32, tag="negmax")
                        nc.vector.tensor_reduce(
                            out=negmax[:], in_=ps[:],
                            axis=mybir.AxisListType.X,
                            op=mybir.AluOpType.max, negate=True)
                        s0t = spool.tile([128, N], F32, tag="s0")
                        sums = spool.tile([128, 1], F32, tag="sums")
                        nc.scalar.activation(
                            out=s0t[:], in_=ps[:],
                            func=mybir.ActivationFunctionType.Exp,
                            bias=negmax[:, 0:1], accum_out=sums[:, 0:1])
                        lnsum = spool.tile([128, 1], F32, tag="lnsum")
                        nc.scalar.activation(
                            out=lnsum[:], in_=sums[:],
                            func=mybir.ActivationFunctionType.Ln)
                        bias2 = spool.tile([128, 1], F32, tag="bias2")
                        nc.vector.tensor_tensor(
                            out=bias2[:], in0=negmax[:], in1=lnsum[:],
                            op=mybir.AluOpType.subtract)
                        nc.scalar.activation(
                            out=s0t[:], in_=ps[:],
                            func=mybir.ActivationFunctionType.Exp,
                            bias=bias2[:, 0:1])
                        nc.sync.dma_start(
                            out=s0_d[g * N + nb * 128:g * N + (nb + 1) * 128, :],
                            in_=s0t[:])

                # -- classifier head on |mean_s - mean_t|
                d = spool.tile([H, GPC], F32, tag="d")
                nc.vector.tensor_tensor(out=d[:], in0=gs_t["s"][:],
                                        in1=gs_t["t"][:],
                                        op=mybir.AluOpType.subtract)
                gabs = spool.tile([H, GPC], F32, tag="gabs")
                nc.scalar.activation(out=gabs[:], in_=d[:],
                                     func=mybir.ActivationFunctionType.Abs,
                                     scale=1.0 / N)
                p1 = pss.tile([H, GPC], F32, tag="s")
                nc.tensor.matmul(out=p1[:], lhsT=wc1[:], rhs=gabs[:],
                                 start=True, stop=True)
                r1 = spool.tile([H, GPC], F32, tag="r1")
                nc.scalar.activation(out=r1[:], in_=p1[:],
                                     func=mybir.ActivationFunctionType.Relu,
                                     bias=bc1[:, 0:1])
                p2 = pss.tile([C, GPC], F32, tag="s")
                nc.tensor.matmul(out=p2[:], lhsT=wc2[:], rhs=r1[:],
                                 start=True, stop=True)
                lg = spool.tile([C, GPC], F32, tag="lg")
                nc.scalar.activation(out=lg[:], in_=p2[:],
                                     func=mybir.ActivationFunctionType.Identity,
                                     bias=bc2[:, 0:1])
                nc.sync.dma_start(out=lg_d[:, :].rearrange("g c -> c g"),
                                  in_=lg[:])

            if nreps == 1:
                body()
            else:
                with tc.For_i(0, nreps, 1):
                    body()

    nc.compile()
    return nc


def _get_nc(nreps=1):
    if nreps not in _BUILT:
        _BUILT[nreps] = _build(nreps)
    return _BUILT[nreps]


# ---------------------------------------------------------------- entry
def kernel(**inputs):
    nc = _get_nc(1)
    in_maps = _host_preprocess(inputs)
    res = bass_utils.run_bass_kernel_spmd(
        nc, in_maps, core_ids=list(range(NCORES)))
    logits = np.concatenate(
        [res.results[c]["lg"] for c in range(NCORES)], axis=0)
    s0 = np.concatenate(
        [res.results[c]["s0"].reshape(GPC, N, N) for c in range(NCORES)],
        axis=0)
    return logits.astype(np.float32), s0.astype(np.float32)


# revision 10
# speedup vs baseline: 180.8981x; 180.8981x over previous
"""TRN2 Bass kernel for nn_CodeSimilarityDetectionModel (GNN message passing).

Model (per graph pair, B=64 pairs, N=1024 nodes/graph, H=IN_DIM=128, C=4):
  h = 3-layer GCN encoder (shared weights) on each side
  S0 = softmax_row(Hs @ Ht^T)              [B, N, N]
  logits = MLP(|mean(Hs) - mean(Ht)|)      [B, C]

Distribution: batch-parallel over 8 NeuronCores, 8 graph pairs per core,
weights replicated, no collectives.

Device algorithm per graph-side:
  - Dense normalized adjacency A^T (with self loops, deg^-1/2 norms and
    duplicate-edge multiplicity baked into per-edge values) is built in
    SBUF as 8 blocks of [128 src x 1024 dst] fp16 via GPSIMD local_scatter
    from host-prepared padded CSR rows.
  - Layers: Y = (h W) in fp32 on PE;  Z^T = sum_m Y[m-blk]^T-matmuls with
    A^T blocks as the fp16 moving operand (PSUM accumulates in fp32);
    h_next^T = relu(Z^T + b) on ACT straight out of PSUM.
  - S0 logits: f32r x f32r matmuls of h3^T blocks; row-softmax via
    (DVE max-reduce, ACT exp with accumulate, two-pass exp with
    bias = -max - ln(sum)).
"""

import sys

sys.path.insert(0, "/opt/trn_rl_repo")

import numpy as np

import concourse.bass as bass
import concourse.mybir as mybir
from concourse import bacc, bass_utils
from concourse.tile import TileContext
from concourse.masks import make_identity

# ---------------------------------------------------------------- constants
B, N, H, IN_DIM, DEG, C = 64, 1024, 128, 128, 16, 4
NCORES = 8
GPC = B // NCORES          # graph pairs per core (8)
NBLK = N // 128            # 8 row blocks per graph
PADW = 64                  # padded CSR width (max row degree; asserted)
F32 = mybir.dt.float32
F32R = mybir.dt.float32r
F16 = mybir.dt.float16
I16 = mybir.dt.int16

_BUILT = {}


# ---------------------------------------------------------------- host prep
def _prep_side(edge_index, core, g):
    """Padded CSR (by src) for graph `core*GPC+g`, with self loops, dup
    multiplicity and symmetric deg^-1/2 normalization baked into values.

    Returns (idx [NBLK,128,PADW] int16 with -1 padding,
             val [NBLK,128,PADW] float16)."""
    bg = core * GPC + g
    e0 = bg * N * DEG
    src = (edge_index[0, e0:e0 + N * DEG] - bg * N).astype(np.int64)
    dst = (edge_index[1, e0:e0 + N * DEG] - bg * N).astype(np.int64)

    loop = np.arange(N, dtype=np.int64)
    dst_all = np.concatenate([dst, loop])
    deg = np.bincount(dst_all, minlength=N).astype(np.float64)
    dinv = (1.0 / np.sqrt(deg)).astype(np.float32)

    key = np.concatenate([src * N + dst, loop * N + loop])
    uniq, counts = np.unique(key, return_counts=True)
    usrc = (uniq // N).astype(np.int64)
    udst = (uniq % N).astype(np.int64)
    vals = counts.astype(np.float32) * dinv[usrc] * dinv[udst]

    starts = np.searchsorted(usrc, np.arange(N))
    pos = np.arange(len(usrc)) - starts[usrc]
    assert pos.max() < PADW, f"row degree {pos.max() + 1} exceeds PADW={PADW}"

    idx = np.full((N, PADW), -1, dtype=np.int16)
    val = np.zeros((N, PADW), dtype=np.float32)
    idx[usrc, pos] = udst.astype(np.int16)
    val[usrc, pos] = vals
    return (idx.reshape(NBLK, 128, PADW),
            val.reshape(NBLK, 128, PADW).astype(np.float16))


def _host_preprocess(inputs):
    """Slice + localize per-core inputs; CSR-ify edges. Returns in_maps."""
    x_s = np.asarray(inputs["x_s"], dtype=np.float32)
    x_t = np.asarray(inputs["x_t"], dtype=np.float32)
    ei_s = np.asarray(inputs["edge_index_s"])
    ei_t = np.asarray(inputs["edge_index_t"])

    weights = {}
    for k in ("W1", "W2", "W3", "Wc1", "Wc2"):
        weights[k] = np.ascontiguousarray(np.asarray(inputs[k], np.float32))
    for k in ("b1", "b2", "b3", "bc1", "bc2"):
        weights[k] = np.ascontiguousarray(np.asarray(inputs[k], np.float32))

    in_maps = []
    for core in range(NCORES):
        r0 = core * GPC * N
        m = dict(weights)
        for name, x in (("s", x_s), ("t", x_t)):
            # [g*N + nb*128 + p, h] -> [p, g, nb, h]: contiguous per-partition
            xr = x[r0:r0 + GPC * N].reshape(GPC, NBLK, 128, IN_DIM)
            m[f"x_{name}"] = np.ascontiguousarray(xr.transpose(2, 0, 1, 3))
        for name, ei in (("s", ei_s), ("t", ei_t)):
            idxs = np.empty((GPC, NBLK, 128, PADW), np.int16)
            vals = np.empty((GPC, NBLK, 128, PADW), np.float16)
            for g in range(GPC):
                idxs[g], vals[g] = _prep_side(ei, core, g)
            m[f"ci_{name}"] = np.ascontiguousarray(idxs.transpose(2, 0, 1, 3))
            m[f"cv_{name}"] = np.ascontiguousarray(vals.transpose(2, 0, 1, 3))
        in_maps.append(m)
    return in_maps


# ---------------------------------------------------------------- device
def _build(nreps=1):
    """Build + compile the per-core Bass program. nreps>1 wraps the body in
    a For_i loop for wall-clock timing (work repeats identically)."""
    nc = bacc.Bacc("TRN2", target_bir_lowering=False, debug=False)

    x_d = {s: nc.dram_tensor(f"x_{s}", [128, GPC, NBLK, IN_DIM], F32,
                             kind="ExternalInput") for s in "st"}
    ci_d = {s: nc.dram_tensor(f"ci_{s}", [128, GPC, NBLK, PADW], I16,
                              kind="ExternalInput") for s in "st"}
    cv_d = {s: nc.dram_tensor(f"cv_{s}", [128, GPC, NBLK, PADW], F16,
                              kind="ExternalInput") for s in "st"}
    W_d = [nc.dram_tensor(n, [H, H], F32, kind="ExternalInput")
           for n in ("W1", "W2", "W3")]
    b_d = [nc.dram_tensor(n, [H], F32, kind="ExternalInput")
           for n in ("b1", "b2", "b3")]
    Wc1_d = nc.dram_tensor("Wc1", [H, H], F32, kind="ExternalInput")
    bc1_d = nc.dram_tensor("bc1", [H], F32, kind="ExternalInput")
    Wc2_d = nc.dram_tensor("Wc2", [H, C], F32, kind="ExternalInput")
    bc2_d = nc.dram_tensor("bc2", [C], F32, kind="ExternalInput")

    s0_d = nc.dram_tensor("s0", [GPC * N, N], F32, kind="ExternalOutput")
    lg_d = nc.dram_tensor("lg", [GPC, C], F32, kind="ExternalOutput")

    with TileContext(nc) as tc:
        import contextlib
        ctx = contextlib.ExitStack()
        with ctx:
            cpool = ctx.enter_context(tc.tile_pool(name="consts", bufs=1))
            apool = ctx.enter_context(tc.tile_pool(name="amat", bufs=1))
            hpool = ctx.enter_context(tc.tile_pool(name="acts", bufs=2))
            iopool = ctx.enter_context(tc.tile_pool(name="io", bufs=2))
            spool = ctx.enter_context(tc.tile_pool(name="smax", bufs=3))
            psxy = ctx.enter_context(
                tc.tile_pool(name="psxy", bufs=2, space="PSUM"))
            psz = ctx.enter_context(
                tc.tile_pool(name="psz", bufs=1, space="PSUM"))
            pss = ctx.enter_context(
                tc.tile_pool(name="pss", bufs=1, space="PSUM"))

            # ---- constants
            ident = cpool.tile([128, 128], F32, tag="ident")
            make_identity(nc, ident[:])
            W_t = []
            b_t = []
            for l in range(3):
                wt = cpool.tile([H, H], F32, tag=f"W{l}")
                nc.sync.dma_start(out=wt[:], in_=W_d[l][:, :])
                W_t.append(wt)
                bt = cpool.tile([H, 1], F32, tag=f"b{l}")
                nc.sync.dma_start(out=bt[:], in_=b_d[l][:, None])
                b_t.append(bt)
            wc1 = cpool.tile([H, H], F32, tag="wc1")
            nc.sync.dma_start(out=wc1[:], in_=Wc1_d[:, :])
            wc2 = cpool.tile([H, C], F32, tag="wc2")
            nc.sync.dma_start(out=wc2[:], in_=Wc2_d[:, :])
            bc1 = cpool.tile([H, 1], F32, tag="bc1")
            nc.sync.dma_start(out=bc1[:], in_=bc1_d[:, None])
            bc2 = cpool.tile([C, 1], F32, tag="bc2")
            nc.sync.dma_start(out=bc2[:], in_=bc2_d[:, None])

            gs_t = {s: cpool.tile([H, GPC], F32, tag=f"gsum{s}",
                                  name=f"gsum_{s}") for s in "st"}

            def body():
                for g in range(GPC):
                    h3 = {}
                    for s in "st":
                        # -- load x rows for this graph, 8 col-blocks
                        xg = iopool.tile([128, NBLK * 128], F32, tag="xg")
                        nc.sync.dma_start(
                            out=xg[:].rearrange("p (nb h) -> p nb h", h=128),
                            in_=x_d[s][:, g, :, :])
                        # -- transpose to x^T [h, n]
                        pxt = psxy.tile([128, NBLK * 128], F32, tag="xy")
                        for nb in range(NBLK):
                            nc.tensor.transpose(
                                out=pxt[:, nb * 128:(nb + 1) * 128],
                                in_=xg[:, nb * 128:(nb + 1) * 128],
                                identity=ident[:])
                        hT = hpool.tile([128, N], F32, tag=f"hT{s}")
                        nc.vector.tensor_copy(out=hT[:], in_=pxt[:])

                        # -- build A^T (normalized, with loops) via scatter
                        amat = apool.tile([128, NBLK * N], F16, tag=f"A{s}")
                        ci = iopool.tile([128, NBLK * PADW], I16, tag="ci")
                        cv = iopool.tile([128, NBLK * PADW], F16, tag="cv")
                        nc.sync.dma_start(
                            out=ci[:].rearrange("p (nb w) -> p nb w", w=PADW),
                            in_=ci_d[s][:, g, :, :])
                        nc.sync.dma_start(
                            out=cv[:].rearrange("p (nb w) -> p nb w", w=PADW),
                            in_=cv_d[s][:, g, :, :])
                        for mb in range(NBLK):
                            nc.gpsimd.local_scatter(
                                out_ap=amat[:, mb * N:(mb + 1) * N],
                                data_ap=cv[:, mb * PADW:(mb + 1) * PADW],
                                idxs_ap=ci[:, mb * PADW:(mb + 1) * PADW],
                                channels=128, num_elems=N, num_idxs=PADW)

                        # -- 3 GCN layers
                        for l in range(3):
                            py = psxy.tile([128, NBLK * 128], F32, tag="xy")
                            for nb in range(NBLK):
                                nc.tensor.matmul(
                                    out=py[:, nb * 128:(nb + 1) * 128],
                                    lhsT=hT[:, nb * 128:(nb + 1) * 128],
                                    rhs=W_t[l][:], start=True, stop=True)
                            ybf = hpool.tile([128, N], F16, tag="ybf")
                            nc.vector.tensor_copy(out=ybf[:], in_=py[:])
                            pz = psz.tile([128, N], F32, tag="z")
                            for mb in range(NBLK):
                                for cchunk in range(2):
                                    cs = slice(cchunk * 512, cchunk * 512 + 512)
                                    nc.tensor.matmul(
                                        out=pz[:, cs],
                                        lhsT=ybf[:, mb * 128:(mb + 1) * 128],
                                        rhs=amat[:, mb * N:(mb + 1) * N][:, cs],
                                        start=(mb == 0), stop=(mb == NBLK - 1))
                            if l < 2:
                                hT = hpool.tile([128, N], F32, tag=f"hT{s}")
                                nc.scalar.activation(
                                    out=hT[:], in_=pz[:],
                                    func=mybir.ActivationFunctionType.Relu,
                                    bias=b_t[l][:, 0:1])
                            else:
                                h3r = hpool.tile([128, N], F32R, tag=f"h3{s}")
                                nc.scalar.activation(
                                    out=h3r[:], in_=pz[:],
                                    func=mybir.ActivationFunctionType.Identity,
                                    bias=b_t[l][:, 0:1])
                                h3[s] = h3r
                        # -- graph embedding accumulation (sum over nodes)
                        nc.vector.tensor_reduce(
                            out=gs_t[s][:, g:g + 1],
                            in_=h3[s][:].bitcast(F32),
                            axis=mybir.AxisListType.X, op=mybir.AluOpType.add)

                    # -- S0 = row-softmax(Hs @ Ht^T) for this pair
                    for nb in range(NBLK):
                        ps = pss.tile([128, N], F32, tag="s")
                        for cchunk in range(2):
                            cs = slice(cchunk * 512, cchunk * 512 + 512)
                            nc.tensor.matmul(
                                out=ps[:, cs],
                                lhsT=h3["s"][:, nb * 128:(nb + 1) * 128],
                                rhs=h3["t"][:, cs],
                                start=True, stop=True)
                        negmax = spool.tile([128, 1], F32, tag="negmax")
                        nc.vector.tensor_reduce(
                            out=negmax[:], in_=ps[:],
                            axis=mybir.AxisListType.X,
                            op=mybir.AluOpType.max, negate=True)
                        s0t = spool.tile([128, N], F32, tag="s0")
                        sums = spool.tile([128, 1], F32, tag="sums")
                        nc.scalar.activation(
                            out=s0t[:], in_=ps[:],
                            func=mybir.ActivationFunctionType.Exp,
                            bias=negmax[:, 0:1], accum_out=sums[:, 0:1])
                        lnsum = spool.tile([128, 1], F32, tag="lnsum")
                        nc.scalar.activation(
                            out=lnsum[:], in_=sums[:],
                            func=mybir.ActivationFunctionType.Ln)
                        bias2 = spool.tile([128, 1], F32, tag="bias2")
                        nc.vector.tensor_tensor(
                            out=bias2[:], in0=negmax[:], in1=lnsum[:],
                            op=mybir.AluOpType.subtract)
                        nc.scalar.activation(
                            out=s0t[:], in_=ps[:],
                            func=mybir.ActivationFunctionType.Exp,
                            bias=bias2[:, 0:1])
                        nc.sync.dma_start(
                            out=s0_d[g * N + nb * 128:g * N + (nb + 1) * 128, :],
                            in_=s0t[:])

                # -- classifier head on |mean_s - mean_t|
                d = spool.tile([H, GPC], F32, tag="d")
                nc.vector.tensor_tensor(out=d[:], in0=gs_t["s"][:],
                                        in1=gs_t["t"][:],
                                        op=mybir.AluOpType.subtract)
                gabs = spool.tile([H, GPC], F32, tag="gabs")
                nc.scalar.activation(out=gabs[:], in_=d[:],
                                     func=mybir.ActivationFunctionType.Abs,
                                     scale=1.0 / N)
                p1 = pss.tile([H, GPC], F32, tag="s")
                nc.tensor.matmul(out=p1[:], lhsT=wc1[:], rhs=gabs[:],
                                 start=True, stop=True)
                r1 = spool.tile([H, GPC], F32, tag="r1")
                nc.scalar.activation(out=r1[:], in_=p1[:],
                                     func=mybir.ActivationFunctionType.Relu,
                                     bias=bc1[:, 0:1])
                p2 = pss.tile([C, GPC], F32, tag="s")
                nc.tensor.matmul(out=p2[:], lhsT=wc2[:], rhs=r1[:],
                                 start=True, stop=True)
                lg = spool.tile([C, GPC], F32, tag="lg")
                nc.scalar.activation(out=lg[:], in_=p2[:],
                                     func=mybir.ActivationFunctionType.Identity,
                                     bias=bc2[:, 0:1])
                nc.sync.dma_start(out=lg_d[:, :].rearrange("g c -> c g"),
                                  in_=lg[:])

            if nreps == 1:
                body()
            else:
                with tc.For_i(0, nreps, 1):
                    body()

    nc.compile()
    return nc


def _get_nc(nreps=1):
    if nreps not in _BUILT:
        _BUILT[nreps] = _build(nreps)
    return _BUILT[nreps]


# ---------------------------------------------------------------- entry
def kernel(**inputs):
    nc = _get_nc(1)
    in_maps = _host_preprocess(inputs)
    res = bass_utils.run_bass_kernel_spmd(
        nc, in_maps, core_ids=list(range(NCORES)))
    logits = np.concatenate(
        [res.results[c]["lg"] for c in range(NCORES)], axis=0)
    s0 = np.concatenate(
        [res.results[c]["s0"].reshape(GPC, N, N) for c in range(NCORES)],
        axis=0)
    return logits.astype(np.float32), s0.astype(np.float32)


# revision 11
# speedup vs baseline: 381.6029x; 2.1095x over previous
"""TRN2 Bass kernel for nn_CodeSimilarityDetectionModel (GNN message passing).

Model (per graph pair, B=64 pairs, N=1024 nodes/graph, H=IN_DIM=128, C=4):
  h = 3-layer GCN encoder (shared weights) on each side
  S0 = softmax_row(Hs @ Ht^T)              [B, N, N]
  logits = MLP(|mean(Hs) - mean(Ht)|)      [B, C]

Distribution: batch-parallel over 8 NeuronCores, 8 graph pairs per core,
weights replicated, no collectives.

Device algorithm per graph-side:
  - Dense normalized adjacency A^T (with self loops, deg^-1/2 norms and
    duplicate-edge multiplicity baked into per-edge values) is built in
    SBUF as 8 blocks of [128 src x 1024 dst] fp16 via GPSIMD local_scatter
    from host-prepared padded CSR rows.
  - Layers: Y = (h W) in fp32 on PE;  Z^T = sum_m Y[m-blk]^T-matmuls with
    A^T blocks as the fp16 moving operand (PSUM accumulates in fp32);
    h_next^T = relu(Z^T + b) on ACT straight out of PSUM.
  - S0 logits: f32r x f32r matmuls of h3^T blocks; row-softmax via
    (DVE max-reduce, ACT exp with accumulate, two-pass exp with
    bias = -max - ln(sum)).
"""

import sys

sys.path.insert(0, "/opt/trn_rl_repo")

import numpy as np

import concourse.bass as bass
import concourse.mybir as mybir
from concourse import bacc, bass_utils
from concourse.tile import TileContext
from concourse.masks import make_identity

# ---------------------------------------------------------------- constants
B, N, H, IN_DIM, DEG, C = 64, 1024, 128, 128, 16, 4
NCORES = 8
GPC = B // NCORES          # graph pairs per core (8)
NBLK = N // 128            # 8 row blocks per graph
PADW = 64                  # padded CSR width (max row degree; asserted)
F32 = mybir.dt.float32
F32R = mybir.dt.float32r
F16 = mybir.dt.float16
I16 = mybir.dt.int16

_BUILT = {}


# ---------------------------------------------------------------- host prep
def _prep_side(edge_index, core, g):
    """Padded CSR (by src) for graph `core*GPC+g`, with self loops, dup
    multiplicity and symmetric deg^-1/2 normalization baked into values.

    Returns (idx [NBLK,128,PADW] int16 with -1 padding,
             val [NBLK,128,PADW] float16)."""
    bg = core * GPC + g
    e0 = bg * N * DEG
    src = (edge_index[0, e0:e0 + N * DEG] - bg * N).astype(np.int64)
    dst = (edge_index[1, e0:e0 + N * DEG] - bg * N).astype(np.int64)

    loop = np.arange(N, dtype=np.int64)
    dst_all = np.concatenate([dst, loop])
    deg = np.bincount(dst_all, minlength=N).astype(np.float64)
    dinv = (1.0 / np.sqrt(deg)).astype(np.float32)

    key = np.concatenate([src * N + dst, loop * N + loop])
    uniq, counts = np.unique(key, return_counts=True)
    usrc = (uniq // N).astype(np.int64)
    udst = (uniq % N).astype(np.int64)
    vals = counts.astype(np.float32) * dinv[usrc] * dinv[udst]

    starts = np.searchsorted(usrc, np.arange(N))
    pos = np.arange(len(usrc)) - starts[usrc]
    assert pos.max() < PADW, f"row degree {pos.max() + 1} exceeds PADW={PADW}"

    idx = np.full((N, PADW), -1, dtype=np.int16)
    val = np.zeros((N, PADW), dtype=np.float32)
    idx[usrc, pos] = udst.astype(np.int16)
    val[usrc, pos] = vals
    return (idx.reshape(NBLK, 128, PADW),
            val.reshape(NBLK, 128, PADW).astype(np.float16))


def _host_preprocess(inputs):
    """Slice + localize per-core inputs; CSR-ify edges. Returns in_maps."""
    x_s = np.asarray(inputs["x_s"], dtype=np.float32)
    x_t = np.asarray(inputs["x_t"], dtype=np.float32)
    ei_s = np.asarray(inputs["edge_index_s"])
    ei_t = np.asarray(inputs["edge_index_t"])

    weights = {}
    for k in ("W1", "W2", "W3", "Wc1", "Wc2"):
        weights[k] = np.ascontiguousarray(np.asarray(inputs[k], np.float32))
    for k in ("b1", "b2", "b3", "bc1", "bc2"):
        weights[k] = np.ascontiguousarray(np.asarray(inputs[k], np.float32))

    in_maps = []
    for core in range(NCORES):
        r0 = core * GPC * N
        m = dict(weights)
        for name, x in (("s", x_s), ("t", x_t)):
            # [g*N + nb*128 + p, h] -> [p, g, nb, h]: contiguous per-partition
            xr = x[r0:r0 + GPC * N].reshape(GPC, NBLK, 128, IN_DIM)
            m[f"x_{name}"] = np.ascontiguousarray(xr.transpose(2, 0, 1, 3))
        for name, ei in (("s", ei_s), ("t", ei_t)):
            idxs = np.empty((GPC, NBLK, 128, PADW), np.int16)
            vals = np.empty((GPC, NBLK, 128, PADW), np.float16)
            for g in range(GPC):
                idxs[g], vals[g] = _prep_side(ei, core, g)
            m[f"ci_{name}"] = np.ascontiguousarray(idxs.transpose(2, 0, 1, 3))
            m[f"cv_{name}"] = np.ascontiguousarray(vals.transpose(2, 0, 1, 3))
        in_maps.append(m)
    return in_maps


# ---------------------------------------------------------------- device
def _build(nreps=1):
    """Build + compile the per-core Bass program. nreps>1 wraps the body in
    a For_i loop for wall-clock timing (work repeats identically)."""
    nc = bacc.Bacc("TRN2", target_bir_lowering=False, debug=False)

    x_d = {s: nc.dram_tensor(f"x_{s}", [128, GPC, NBLK, IN_DIM], F32,
                             kind="ExternalInput") for s in "st"}
    ci_d = {s: nc.dram_tensor(f"ci_{s}", [128, GPC, NBLK, PADW], I16,
                              kind="ExternalInput") for s in "st"}
    cv_d = {s: nc.dram_tensor(f"cv_{s}", [128, GPC, NBLK, PADW], F16,
                              kind="ExternalInput") for s in "st"}
    W_d = [nc.dram_tensor(n, [H, H], F32, kind="ExternalInput")
           for n in ("W1", "W2", "W3")]
    b_d = [nc.dram_tensor(n, [H], F32, kind="ExternalInput")
           for n in ("b1", "b2", "b3")]
    Wc1_d = nc.dram_tensor("Wc1", [H, H], F32, kind="ExternalInput")
    bc1_d = nc.dram_tensor("bc1", [H], F32, kind="ExternalInput")
    Wc2_d = nc.dram_tensor("Wc2", [H, C], F32, kind="ExternalInput")
    bc2_d = nc.dram_tensor("bc2", [C], F32, kind="ExternalInput")

    s0_d = nc.dram_tensor("s0", [GPC * N, N], F32, kind="ExternalOutput")
    lg_d = nc.dram_tensor("lg", [GPC, C], F32, kind="ExternalOutput")

    with TileContext(nc) as tc:
        import contextlib
        ctx = contextlib.ExitStack()
        with ctx:
            cpool = ctx.enter_context(tc.tile_pool(name="consts", bufs=1))
            apool = ctx.enter_context(tc.tile_pool(name="amat", bufs=2))
            hpool = ctx.enter_context(tc.tile_pool(name="acts", bufs=3))
            iopool = ctx.enter_context(tc.tile_pool(name="io", bufs=2))
            spool = ctx.enter_context(tc.tile_pool(name="smax", bufs=3))
            psxy = ctx.enter_context(
                tc.tile_pool(name="psxy", bufs=1, space="PSUM"))
            psz = ctx.enter_context(
                tc.tile_pool(name="psz", bufs=1, space="PSUM"))
            pss = ctx.enter_context(
                tc.tile_pool(name="pss", bufs=2, space="PSUM"))

            # ---- constants
            ident = cpool.tile([128, 128], F32, tag="ident")
            make_identity(nc, ident[:])
            W_t = []
            b_t = []
            for l in range(3):
                wt = cpool.tile([H, H], F32, tag=f"W{l}")
                nc.sync.dma_start(out=wt[:], in_=W_d[l][:, :])
                W_t.append(wt)
                bt = cpool.tile([H, 1], F32, tag=f"b{l}")
                nc.sync.dma_start(out=bt[:], in_=b_d[l][:, None])
                b_t.append(bt)
            wc1 = cpool.tile([H, H], F32, tag="wc1")
            nc.sync.dma_start(out=wc1[:], in_=Wc1_d[:, :])
            wc2 = cpool.tile([H, C], F32, tag="wc2")
            nc.sync.dma_start(out=wc2[:], in_=Wc2_d[:, :])
            bc1 = cpool.tile([H, 1], F32, tag="bc1")
            nc.sync.dma_start(out=bc1[:], in_=bc1_d[:, None])
            bc2 = cpool.tile([C, 1], F32, tag="bc2")
            nc.sync.dma_start(out=bc2[:], in_=bc2_d[:, None])

            gs_t = {s: cpool.tile([H, GPC], F32, tag=f"gsum{s}",
                                  name=f"gsum_{s}") for s in "st"}

            def body():
                for g in range(GPC):
                    h3 = {}
                    for s in "st":
                        # -- load x rows for this graph, 8 col-blocks
                        xg = iopool.tile([128, NBLK * 128], F32, tag="xg")
                        nc.sync.dma_start(
                            out=xg[:].rearrange("p (nb h) -> p nb h", h=128),
                            in_=x_d[s][:, g, :, :])
                        # -- transpose to x^T [h, n]
                        pxt = psxy.tile([128, NBLK * 128], F32, tag="xy")
                        for nb in range(NBLK):
                            nc.tensor.transpose(
                                out=pxt[:, nb * 128:(nb + 1) * 128],
                                in_=xg[:, nb * 128:(nb + 1) * 128],
                                identity=ident[:])
                        hT = hpool.tile([128, N], F32, tag=f"hT{s}")
                        nc.vector.tensor_copy(out=hT[:], in_=pxt[:])

                        # -- build A^T (normalized, with loops) via scatter
                        amat = apool.tile([128, NBLK * N], F16, tag=f"A{s}")
                        ci = iopool.tile([128, NBLK * PADW], I16, tag="ci")
                        cv = iopool.tile([128, NBLK * PADW], F16, tag="cv")
                        nc.sync.dma_start(
                            out=ci[:].rearrange("p (nb w) -> p nb w", w=PADW),
                            in_=ci_d[s][:, g, :, :])
                        nc.sync.dma_start(
                            out=cv[:].rearrange("p (nb w) -> p nb w", w=PADW),
                            in_=cv_d[s][:, g, :, :])
                        for mb in range(NBLK):
                            nc.gpsimd.local_scatter(
                                out_ap=amat[:, mb * N:(mb + 1) * N],
                                data_ap=cv[:, mb * PADW:(mb + 1) * PADW],
                                idxs_ap=ci[:, mb * PADW:(mb + 1) * PADW],
                                channels=128, num_elems=N, num_idxs=PADW)

                        # -- 3 GCN layers
                        for l in range(3):
                            py = psxy.tile([128, NBLK * 128], F32, tag="xy")
                            for nb in range(NBLK):
                                nc.tensor.matmul(
                                    out=py[:, nb * 128:(nb + 1) * 128],
                                    lhsT=hT[:, nb * 128:(nb + 1) * 128],
                                    rhs=W_t[l][:], start=True, stop=True)
                            ybf = hpool.tile([128, N], F16, tag="ybf")
                            nc.vector.tensor_copy(out=ybf[:], in_=py[:])
                            pz = psz.tile([128, N], F32, tag="z")
                            for mb in range(NBLK):
                                for cchunk in range(2):
                                    cs = slice(cchunk * 512, cchunk * 512 + 512)
                                    nc.tensor.matmul(
                                        out=pz[:, cs],
                                        lhsT=ybf[:, mb * 128:(mb + 1) * 128],
                                        rhs=amat[:, mb * N:(mb + 1) * N][:, cs],
                                        start=(mb == 0), stop=(mb == NBLK - 1))
                            if l < 2:
                                hT = hpool.tile([128, N], F32, tag=f"hT{s}")
                                nc.scalar.activation(
                                    out=hT[:], in_=pz[:],
                                    func=mybir.ActivationFunctionType.Relu,
                                    bias=b_t[l][:, 0:1])
                            else:
                                h3r = hpool.tile([128, N], F32R, tag=f"h3{s}")
                                nc.scalar.activation(
                                    out=h3r[:], in_=pz[:],
                                    func=mybir.ActivationFunctionType.Identity,
                                    bias=b_t[l][:, 0:1])
                                h3[s] = h3r
                        # -- graph embedding accumulation (sum over nodes)
                        nc.vector.tensor_reduce(
                            out=gs_t[s][:, g:g + 1],
                            in_=h3[s][:].bitcast(F32),
                            axis=mybir.AxisListType.X, op=mybir.AluOpType.add)

                    # -- S0 = row-softmax(Hs @ Ht^T) for this pair
                    for nb in range(NBLK):
                        ps = pss.tile([128, N], F32, tag="s")
                        for cchunk in range(2):
                            cs = slice(cchunk * 512, cchunk * 512 + 512)
                            nc.tensor.matmul(
                                out=ps[:, cs],
                                lhsT=h3["s"][:, nb * 128:(nb + 1) * 128],
                                rhs=h3["t"][:, cs],
                                start=True, stop=True)
                        negmax = spool.tile([128, 1], F32, tag="negmax")
                        nc.vector.tensor_reduce(
                            out=negmax[:], in_=ps[:],
                            axis=mybir.AxisListType.X,
                            op=mybir.AluOpType.max, negate=True)
                        s0t = spool.tile([128, N], F32, tag="s0")
                        sums = spool.tile([128, 1], F32, tag="sums")
                        nc.scalar.activation(
                            out=s0t[:], in_=ps[:],
                            func=mybir.ActivationFunctionType.Exp,
                            bias=negmax[:, 0:1], accum_out=sums[:, 0:1])
                        lnsum = spool.tile([128, 1], F32, tag="lnsum")
                        nc.scalar.activation(
                            out=lnsum[:], in_=sums[:],
                            func=mybir.ActivationFunctionType.Ln)
                        bias2 = spool.tile([128, 1], F32, tag="bias2")
                        nc.vector.tensor_tensor(
                            out=bias2[:], in0=negmax[:], in1=lnsum[:],
                            op=mybir.AluOpType.subtract)
                        nc.scalar.activation(
                            out=s0t[:], in_=ps[:],
                            func=mybir.ActivationFunctionType.Exp,
                            bias=bias2[:, 0:1])
                        nc.sync.dma_start(
                            out=s0_d[g * N + nb * 128:g * N + (nb + 1) * 128, :],
                            in_=s0t[:])

                # -- classifier head on |mean_s - mean_t|
                d = spool.tile([H, GPC], F32, tag="d")
                nc.vector.tensor_tensor(out=d[:], in0=gs_t["s"][:],
                                        in1=gs_t["t"][:],
                                        op=mybir.AluOpType.subtract)
                gabs = spool.tile([H, GPC], F32, tag="gabs")
                nc.scalar.activation(out=gabs[:], in_=d[:],
                                     func=mybir.ActivationFunctionType.Abs,
                                     scale=1.0 / N)
                p1 = pss.tile([H, GPC], F32, tag="s")
                nc.tensor.matmul(out=p1[:], lhsT=wc1[:], rhs=gabs[:],
                                 start=True, stop=True)
                r1 = spool.tile([H, GPC], F32, tag="r1")
                nc.scalar.activation(out=r1[:], in_=p1[:],
                                     func=mybir.ActivationFunctionType.Relu,
                                     bias=bc1[:, 0:1])
                p2 = pss.tile([C, GPC], F32, tag="s")
                nc.tensor.matmul(out=p2[:], lhsT=wc2[:], rhs=r1[:],
                                 start=True, stop=True)
                lg = spool.tile([C, GPC], F32, tag="lg")
                nc.scalar.activation(out=lg[:], in_=p2[:],
                                     func=mybir.ActivationFunctionType.Identity,
                                     bias=bc2[:, 0:1])
                nc.sync.dma_start(out=lg_d[:, :].rearrange("g c -> c g"),
                                  in_=lg[:])

            if nreps == 1:
                body()
            else:
                with tc.For_i(0, nreps, 1):
                    body()

    nc.compile()
    return nc


def _get_nc(nreps=1):
    if nreps not in _BUILT:
        _BUILT[nreps] = _build(nreps)
    return _BUILT[nreps]


# ---------------------------------------------------------------- entry
def kernel(**inputs):
    nc = _get_nc(1)
    in_maps = _host_preprocess(inputs)
    res = bass_utils.run_bass_kernel_spmd(
        nc, in_maps, core_ids=list(range(NCORES)))
    logits = np.concatenate(
        [res.results[c]["lg"] for c in range(NCORES)], axis=0)
    s0 = np.concatenate(
        [res.results[c]["s0"].reshape(GPC, N, N) for c in range(NCORES)],
        axis=0)
    return logits.astype(np.float32), s0.astype(np.float32)
